# revision 2
# baseline (speedup 1.0000x reference)
"""Trainium2 Bass kernel for nn_Decoder_27848567948011.

Sequential LSTM-style decoder, 40 steps, batch 8192, data-parallel over 8 cores.
Feature-major on-chip layout: activations stored as [feature, batch_tile].

Scale folding (avoids explicit sigmoid; single ACT table set `exp_and_others`):
  sigmoid(x) = 0.5*(tanh(x/2)+1)
  stored TF = tanh(f/2) etc.;  C~ = 2*c_new;  H~ = 2*h_new;  D~ = lam~*C~ = gated carry
  U = (TF+1)*D~_prev ; V = (TI+1)*G ; C~ = U+V ; H~ = (TO+1)*tanh(C~/2)
  lam~ = intensity/2; weights consuming h_new or c_new are pre-halved on host.

Self-contained: hardcodes shapes; reads nothing from the problem directory.
"""
import numpy as np

B, H, F, Z, T = 8192, 256, 128, 64, 43
STEPS = T - 3
N_CORES = 8
NBC = B // N_CORES          # batch per core (1024)
NT = 2                      # batch tiles per core
NB = NBC // NT              # batch tile size (512)
HC = H // 128               # partition chunks for 256-dim (2)

DT_MM = "float32"           # matmul dtype knob: "float32" | "bfloat16"
REPEAT = 1                  # >1: run the whole scan REPEAT times (timing builds)
_cache = {}


def _build_kernel(dt_mm_name: str, repeat: int):
    import concourse.bass as bass
    import concourse.tile as tile
    from concourse import bacc, mybir

    f32 = mybir.dt.float32
    dt_mm = getattr(mybir.dt, dt_mm_name)
    AF = mybir.ActivationFunctionType
    OP = mybir.AluOpType
    need_cast = dt_mm != f32

    nc = bacc.Bacc("TRN2", target_bir_lowering=False, debug=False, num_devices=N_CORES)

    din = {}
    def inp(name, shape, dt):
        din[name] = nc.dram_tensor(name, shape, dt, kind="ExternalInput")
        return din[name]

    epsT = inp("epsT", (STEPS, 128, NBC), f32)      # [eps_inf; eps_prior] feature-major
    lamT = inp("lamT", (STEPS, NBC), f32)           # lam~ = intensity/2
    Wxz = inp("Wxz", (Z, 4 * H), dt_mm)             # Wx rows 0:64
    Wxy = inp("Wxy", (F, 4 * H), dt_mm)             # Wx rows 320:448
    Whh = inp("Whh", (H, 4 * H), dt_mm)             # Wh / 2
    w1h = inp("w1h", (H, F), dt_mm)                 # w1 / 2
    w2d = inp("w2", (F, F), dt_mm)
    w3d = inp("w3", (F, F), dt_mm)
    W47c = inp("W47c", (H, 128), dt_mm)             # [w4_c | w7_c] / 2
    W4y = inp("W4y", (F, Z), dt_mm)                 # w4 rows 512:640
    W47yp = inp("W47yp", (F, 128), dt_mm)           # [w4_yp | w7_yp]
    W58 = inp("W58", (128, 128), dt_mm)             # blockdiag(w5, w8)
    W69 = inp("W69", (128, 128), dt_mm)             # blockdiag(w6, w9)
    ident = inp("ident", (128, 128), dt_mm)
    baseg = inp("baseg", (4 * H, NBC), dt_mm)       # (h_i@Wx[64:320] + b_lstm).T
    base47 = inp("base47", (128, NBC), dt_mm)       # (h_i@[w4_h|w7_h] + [b4|b7]).T
    bvec = inp("bvec", (128, 5), f32)               # b1|b2|b3|[b5;b8]|0.5*[b6;b9]

    dys = nc.dram_tensor("ys", (STEPS, F, NBC), f32, kind="ExternalOutput")
    dmeans = nc.dram_tensor("means", (STEPS, Z, NBC), f32, kind="ExternalOutput")
    dlvs = nc.dram_tensor("lvs", (STEPS, Z, NBC), f32, kind="ExternalOutput")
    dzs = nc.dram_tensor("zs", (STEPS, Z, NBC), f32, kind="ExternalOutput")
    dzps = nc.dram_tensor("zps", (STEPS, Z, NBC), f32, kind="ExternalOutput")

    with tile.TileContext(nc) as tc:
        with (
            tc.tile_pool(name="wpool", bufs=1) as wpool,
            tc.tile_pool(name="gates", bufs=1) as gpool,
            tc.tile_pool(name="cellp", bufs=1) as cpool,
            tc.tile_pool(name="carry", bufs=2) as carry,
            tc.tile_pool(name="iop", bufs=3) as iop,
            tc.tile_pool(name="psum", bufs=1, space="PSUM") as psum,
        ):
            def load(name):
                d = din[name]
                t = wpool.tile(list(d.shape), d.dtype, name=f"sb_{name}")
                nc.sync.dma_start(t[:], d[:])
                return t

            def load_rows(name, nchunk):
                d = din[name]
                ts = []
                for k in range(nchunk):
                    t = wpool.tile([128, d.shape[1]], d.dtype, name=f"sb_{name}{k}")
                    nc.sync.dma_start(t[:], d[k * 128:(k + 1) * 128, :])
                    ts.append(t)
                return ts

            s_Wxz = load("Wxz")
            s_Wxy = load("Wxy")
            s_Wh = load_rows("Whh", HC)
            s_w1 = load_rows("w1h", HC)
            s_w2 = load("w2")
            s_w3 = load("w3")
            s_W47c = load_rows("W47c", HC)
            s_W4y = load("W4y")
            s_W47yp = load("W47yp")
            s_W58 = load("W58")
            s_W69 = load("W69")
            s_I = load("ident")
            s_base = load_rows("baseg", 8)
            s_base47 = load("base47")
            s_bv = load("bvec")

            z_prev = [None] * NT
            y_prev = [None] * NT
            h_prev = [[None] * HC for _ in range(NT)]
            d_prev = [[None] * HC for _ in range(NT)]

            for rep in range(repeat):
              for j in range(STEPS):
                first = (j == 0)
                for nt in range(NT):
                    ns = slice(nt * NB, (nt + 1) * NB)
                    eps = iop.tile([128, NB], f32, name="eps", tag=f"eps{nt}")
                    nc.sync.dma_start(eps[:], epsT[j, :, ns])
                    lam = iop.tile([128, NB], f32, name="lam", tag=f"lam{nt}")
                    lrow = lamT[j, ns]
                    lam_src = bass.AP(tensor=lrow.tensor, offset=lrow.offset,
                                      ap=[[0, 128]] + list(lrow.ap))
                    nc.sync.dma_start(lam[:], lam_src)

                    # ---- gates (order i f g o along 4H) ----
                    gtiles = []
                    for m in range(8):
                        if not first:
                            ps = psum.tile([128, NB], f32, name="gps", tag="gps", bufs=3)
                            mm = slice(m * 128, (m + 1) * 128)
                            nc.tensor.matmul(ps[:], s_Wxz[:, mm], z_prev[nt][0:Z, :],
                                             start=True, stop=False)
                            nc.tensor.matmul(ps[:], s_Wxy[:, mm], y_prev[nt][:],
                                             start=False, stop=False)
                            for k in range(HC):
                                nc.tensor.matmul(ps[:], s_Wh[k][:, mm],
                                                 h_prev[nt][k][:],
                                                 start=False, stop=False)
                            nc.tensor.matmul(ps[:], s_I[:], s_base[m][:, ns],
                                             start=False, stop=True)
                            src = ps
                        else:
                            src = s_base[m]
                        g = gpool.tile([128, NB], f32, name="g", tag=f"g{m}_{nt}")
                        scale = 1.0 if m in (4, 5) else 0.5
                        sap = src[:] if not first else src[:, ns]
                        nc.scalar.activation(g[:], sap, AF.Tanh, scale=scale)
                        gtiles.append(g)
                    TI = gtiles[0:2]; TF = gtiles[2:4]; TG = gtiles[4:6]; TO = gtiles[6:8]

                    # ---- cell ----
                    c_t, h_t, d_t = [None] * HC, [None] * HC, [None] * HC
                    for c in range(HC):
                        # V = (TI+1)*G, written onto TI (dead after)
                        nc.vector.scalar_tensor_tensor(
                            TI[c][:], TI[c][:], 1.0, TG[c][:], op0=OP.add, op1=OP.mult)
                        C = TI[c]
                        if not first:
                            # U = (TF+1)*D_prev onto TF, then C = U+V onto TI
                            nc.vector.scalar_tensor_tensor(
                                TF[c][:], TF[c][:], 1.0, d_prev[nt][c][:],
                                op0=OP.add, op1=OP.mult)
                            nc.vector.tensor_add(C[:], C[:], TF[c][:])
                        tc_t = cpool.tile([128, NB], f32, name="tct", tag=f"tct{c}_{nt}")
                        nc.scalar.activation(tc_t[:], C[:], AF.Tanh, scale=0.5)
                        Ht = carry.tile([128, NB], dt_mm, name="Ht", tag=f"Ht{c}_{nt}")
                        nc.vector.scalar_tensor_tensor(
                            Ht[:], TO[c][:], 1.0, tc_t[:], op0=OP.add, op1=OP.mult)
                        Dt = carry.tile([128, NB], f32, name="Dt", tag=f"Dt{c}_{nt}")
                        nc.vector.tensor_mul(Dt[:], lam[:], C[:])
                        c_t[c], h_t[c], d_t[c] = C, Ht, Dt

                    if need_cast:
                        c_mm = []
                        for c in range(HC):
                            cc = cpool.tile([128, NB], dt_mm, name="cmm",
                                            tag=f"cmm{c}_{nt}")
                            nc.vector.tensor_copy(cc[:], c_t[c][:])
                            c_mm.append(cc)
                    else:
                        c_mm = c_t

                    # ---- y chain ----
                    ps1 = psum.tile([128, NB], f32, name="ps1", tag="sps", bufs=4)
                    for k in range(HC):
                        nc.tensor.matmul(ps1[:], s_w1[k][:], h_t[k][:],
                                         start=(k == 0), stop=(k == HC - 1))
                    y1 = cpool.tile([128, NB], dt_mm, name="y1", tag=f"y1{nt}")
                    nc.scalar.activation(y1[:], ps1[:], AF.Relu, bias=s_bv[:, 0:1])
                    ps2 = psum.tile([128, NB], f32, name="ps2", tag="sps", bufs=4)
                    nc.tensor.matmul(ps2[:], s_w2[:], y1[:], start=True, stop=True)
                    y2 = cpool.tile([128, NB], dt_mm, name="y2", tag=f"y2{nt}")
                    nc.scalar.activation(y2[:], ps2[:], AF.Relu, bias=s_bv[:, 1:2])
                    ps3 = psum.tile([128, NB], f32, name="ps3", tag="sps", bufs=4)
                    nc.tensor.matmul(ps3[:], s_w3[:], y2[:], start=True, stop=True)
                    y_f32 = carry.tile([128, NB], f32, name="yf", tag=f"yf{nt}")
                    nc.scalar.activation(y_f32[:], ps3[:], AF.Relu, bias=s_bv[:, 2:3])
                    nc.sync.dma_start(dys[j, :, ns], y_f32[:])
                    if need_cast:
                        y_t = carry.tile([128, NB], dt_mm, name="ymm", tag=f"ymm{nt}")
                        nc.vector.tensor_copy(y_t[:], y_f32[:])
                    else:
                        y_t = y_f32

                    # ---- inference + prior ----
                    ips = psum.tile([128, NB], f32, name="ips", tag="sps", bufs=4)
                    for k in range(HC):
                        nc.tensor.matmul(ips[:], s_W47c[k][:], c_mm[k][:],
                                         start=(k == 0), stop=False)
                    nc.tensor.matmul(ips[0:Z, :], s_W4y[:], y_t[:],
                                     start=False, stop=False)
                    if not first:
                        nc.tensor.matmul(ips[:], s_W47yp[:], y_prev[nt][:],
                                         start=False, stop=False)
                    nc.tensor.matmul(ips[:], s_I[:], s_base47[:, ns],
                                     start=False, stop=True)
                    hz = cpool.tile([128, NB], dt_mm, name="hz", tag=f"hz{nt}")
                    nc.scalar.activation(hz[:], ips[:], AF.Relu)

                    mps = psum.tile([128, NB], f32, name="mps", tag="sps", bufs=4)
                    nc.tensor.matmul(mps[:], s_W58[:], hz[:], start=True, stop=True)
                    mean = cpool.tile([128, NB], f32, name="mean", tag=f"mean{nt}")
                    nc.scalar.activation(mean[:], mps[:], AF.Relu, bias=s_bv[:, 3:4])
                    nc.sync.dma_start(dmeans[j, :, ns], mean[0:Z, :])

                    lps = psum.tile([128, NB], f32, name="lps", tag="sps", bufs=4)
                    nc.tensor.matmul(lps[:], s_W69[:], hz[:], start=True, stop=True)
                    lv_out = cpool.tile([Z, NB], f32, name="lvo", tag=f"lvo{nt}")
                    nc.scalar.activation(lv_out[:], lps[0:Z, :], AF.Relu,
                                         bias=s_bv[0:Z, 3:4] if False else 0.0)
                    nc.sync.dma_start(dlvs[j, :, ns], lv_out[:])
                    s_t = cpool.tile([128, NB], f32, name="st", tag=f"st{nt}")
                    nc.scalar.activation(s_t[:], lps[:], AF.Exp, scale=0.5,
                                         bias=s_bv[:, 4:5])

                    zt = carry.tile([128, NB], f32, name="zt", tag=f"zt{nt}")
                    nc.vector.scalar_tensor_tensor(
                        zt[:], s_t[:], 1.0, eps[:], op0=OP.max, op1=OP.mult)
                    nc.vector.tensor_add(zt[:], zt[:], mean[:])
                    nc.sync.dma_start(dzs[j, :, ns], zt[0:Z, :])
                    nc.sync.dma_start(dzps[j, :, ns], zt[Z:128, :])
                    if need_cast:
                        zmm = carry.tile([Z, NB], dt_mm, name="zmm", tag=f"zmm{nt}")
                        nc.vector.tensor_copy(zmm[:], zt[0:Z, :])
                        z_prev[nt] = zmm
                    else:
                        z_prev[nt] = zt
                    y_prev[nt] = y_t
                    h_prev[nt] = h_t
                    d_prev[nt] = d_t

    nc.compile()
    return nc


def _prep_inputs(inputs, dt_mm_name):
    import ml_dtypes
    cast = (lambda a: np.ascontiguousarray(a, np.float32)) if dt_mm_name == "float32" \
        else (lambda a: np.ascontiguousarray(a, np.float32).astype(ml_dtypes.bfloat16))

    h_i = np.asarray(inputs["h_i"], np.float32)
    input_t = np.asarray(inputs["input_t"], np.float32)
    eps_inf = np.asarray(inputs["eps_inf"], np.float32)
    eps_prior = np.asarray(inputs["eps_prior"], np.float32)
    Wx = np.asarray(inputs["Wx"], np.float32)
    Wh = np.asarray(inputs["Wh"], np.float32)
    b_lstm = np.asarray(inputs["b_lstm"], np.float32)
    ws = {i: np.asarray(inputs[f"w{i}"], np.float32) for i in range(1, 10)}
    bs = {i: np.asarray(inputs[f"b{i}"], np.float32) for i in range(1, 10)}
    alpha = float(np.asarray(inputs["alpha"]).reshape(-1)[0])
    beta = float(np.asarray(inputs["beta"]).reshape(-1)[0])
    mu0 = float(np.asarray(inputs["mu0"]).reshape(-1)[0])

    # lam~[b, j] = 0.5*(mu0 + alpha*beta*sum_{k<j+3} exp(t_k - t_{j+3}))
    t = input_t
    j3 = np.arange(STEPS) + 3
    diff = t[:, None, :] - t[:, j3][:, :, None]               # [B, STEPS, T]
    mask = np.arange(T)[None, None, :] < j3[None, :, None]
    trig = np.where(mask, np.exp(diff.astype(np.float32)), 0.0).sum(axis=2)
    lam = 0.5 * (mu0 + alpha * beta * trig)
    lamT_full = np.ascontiguousarray(lam.T.astype(np.float32))  # [STEPS, B]

    W58 = np.zeros((128, 128), np.float32)
    W58[0:Z, 0:Z] = ws[5]; W58[Z:, Z:] = ws[8]
    W69 = np.zeros((128, 128), np.float32)
    W69[0:Z, 0:Z] = ws[6]; W69[Z:, Z:] = ws[9]

    baseg_full = (h_i @ Wx[Z:Z + H] + b_lstm).T.astype(np.float32)       # [4H, B]
    base47_full = (h_i @ np.concatenate([ws[4][0:H], ws[7][0:H]], axis=1)
                   + np.concatenate([bs[4], bs[7]])).T.astype(np.float32)

    bvec = np.zeros((128, 5), np.float32)
    bvec[:, 0] = bs[1]; bvec[:, 1] = bs[2]; bvec[:, 2] = bs[3]
    bvec[:, 3] = np.concatenate([bs[5], bs[8]])
    bvec[:, 4] = 0.5 * np.concatenate([bs[6], bs[9]])

    shared = dict(
        Wxz=cast(Wx[0:Z]),
        Wxy=cast(Wx[Z + H:]),
        Whh=cast(0.5 * Wh),
        w1h=cast(0.5 * ws[1]),
        w2=cast(ws[2]), w3=cast(ws[3]),
        W47c=cast(0.5 * np.concatenate([ws[4][H:2 * H], ws[7][H:2 * H]], axis=1)),
        W4y=cast(ws[4][2 * H:2 * H + F]),
        W47yp=cast(np.concatenate([ws[4][2 * H + F:], ws[7][2 * H:]], axis=1)),
        W58=cast(W58), W69=cast(W69),
        ident=cast(np.eye(128, dtype=np.float32)),
        bvec=bvec,
    )
    eps_full = np.concatenate(
        [eps_inf.transpose(0, 2, 1), eps_prior.transpose(0, 2, 1)], axis=1)  # [S,128,B]

    in_maps = []
    for c in range(N_CORES):
        bsl = slice(c * NBC, (c + 1) * NBC)
        m = dict(shared)
        m["epsT"] = np.ascontiguousarray(eps_full[:, :, bsl])
        m["lamT"] = np.ascontiguousarray(lamT_full[:, bsl])
        m["baseg"] = cast(baseg_full[:, bsl])
        m["base47"] = cast(base47_full[:, bsl])
        in_maps.append(m)
    return in_maps


def _assemble(per_core_outs):
    outs = []
    for name in ["ys", "means", "lvs", "zs", "zps"]:
        full = np.concatenate([o[name] for o in per_core_outs], axis=2)  # [S, F, B]
        outs.append(np.ascontiguousarray(full.transpose(2, 0, 1)))
    return tuple(outs)


def get_nc():
    key = (DT_MM, REPEAT)
    if key not in _cache:
        _cache[key] = _build_kernel(DT_MM, REPEAT)
    return _cache[key]


def kernel(**inputs):
    from concourse.bass_utils import run_bass_kernel_spmd
    nc = get_nc()
    in_maps = _prep_inputs(inputs, DT_MM)
    res = run_bass_kernel_spmd(nc, in_maps, core_ids=list(range(N_CORES)))
    return _assemble(res.results)
